# revision 44
# baseline (speedup 1.0000x reference)
"""Trainium2 Bass kernel for nn_EEGMI_RWKV_ResNet_Model — single-core version.

Why one core: the per-exec metric (pipelined dispatch slope) carries
~1.3 ms of client/axon dispatch overhead PER DEVICE, serialized, for any
multi-device round — an empty 8-core kernel measures ~7.7 ms/exec. A
single-device shard_map dispatch pipelines with ~zero marginal overhead,
so the slope equals true device time. We therefore run all 32 batches on
core 0 and minimize device time.

Device-time design (per group of 4 batches, 8 groups streamed):
  - band conv on PE (depthwise as sparse 64->128 matmuls), attention scale
    fused into the psum-drain ACT (bias/scale APs), writing fp8 "Q16"
    (16x-scaled) activations.
  - resnet convs as fp8e4m3 DoubleRow matmuls: F tiles are (128, 3q, TF)
    so the (q0,q1) K-tile pair is one [K,2,N] AP; weights prepacked to
    match. Weights/activations are 16x-scaled into fp8's normal range;
    the 1/16 is folded into the psum drain.
  - rwkv: bf16 matmuls; elementwise work spread across DVE (2x/4x modes),
    ACT, and Pool (gpsimd) engines; the wkv scan is tensor_tensor_scan.
  - LayerNorm over the partition axis: sums via ones(1/H) matmuls,
    inv = ACT Rsqrt, per-t scalars broadcast over partitions via K=1
    matmuls with the gain vector as lhsT.
"""
import os
import numpy as np
import ml_dtypes

import concourse.bass as bass
import concourse.bacc as bacc
import concourse.tile as tile
from concourse import mybir
from concourse.bass_utils import run_bass_kernel_spmd

EPS = 1e-5
B, T, C = 32, 2048, 64
NB, C5, H, L, NBLK, NCLS = 5, 320, 128, 3, 2, 4
NCORE = 1
NGROUP = 16
BL = 2          # batches per group
NCH = 4
CH = 512
TP = T + 4      # padded width for band conv input
TF = T + 4      # conv tensor plane width (data cols 2..2050)
SQ = 16.0       # fp8 "Q16" scale for conv weights/activations

PERM = np.array([(o % 64) * 5 + (o // 64) for o in range(C5)], dtype=np.int64)

F32 = mybir.dt.float32
F32R = mybir.dt.float32r
BF16 = mybir.dt.bfloat16
FP8 = mybir.dt.float8e4
AF = mybir.ActivationFunctionType
ALU = mybir.AluOpType
DR = mybir.MatmulPerfMode.DoubleRow
bf16np = ml_dtypes.bfloat16
fp8np = ml_dtypes.float8_e4m3


# ---------------------------------------------------------------------------
# host-side weight preprocessing (numpy only)
# ---------------------------------------------------------------------------

def _prep_weights(inp):
    f32 = np.float32
    out = {}

    # band conv lhsT: (128, 3m, 3tg, 128) bf16.  out channel o' = j*64 + c
    # (j band, c channel); m block covers j = 2m, 2m+1 (m=2: j=4 only).
    # Tap pairs (0,1), (2,3), (4,-) packed along K: the x tile holds x in
    # partitions 0-63 and x shifted left by one column in partitions 64-127,
    # so tap 2tg sits in rows 0-63 and tap 2tg+1 in rows 64-127.
    bw = np.asarray(inp['band_w'], f32)[:, 0, :]   # (C5, 5) original order
    band_lhsT = np.zeros((128, 3, 3, 128), f32)
    for c in range(64):
        for j in range(NB):
            m, half = divmod(j, 2)
            for k in range(5):
                band_lhsT[(k % 2) * 64 + c, m, k // 2, half * 64 + c] = \
                    bw[c * 5 + j, k]
    out['band_lhsT'] = band_lhsT.astype(bf16np)

    bb = np.asarray(inp['band_b'], f32)[PERM]      # (320,) new order
    bb_pad = np.zeros((384,), f32)
    bb_pad[:C5] = bb

    # pooled-attention coefficients (same trick as before: pooled mean of the
    # band output equals an affine function of per-channel x sums + edge
    # corrections).
    bw_raw = bw.reshape(C, NB, 5)
    denom = f32(1.0 / (NB * T))
    A = bw_raw.sum(axis=(1, 2)) * denom
    E0 = -(bw_raw[:, :, 3] + bw_raw[:, :, 4]).sum(1) * denom
    E1 = -(bw_raw[:, :, 4]).sum(1) * denom
    E2 = -(bw_raw[:, :, 0]).sum(1) * denom
    E3 = -(bw_raw[:, :, 0] + bw_raw[:, :, 1]).sum(1) * denom
    Bb = np.asarray(inp['band_b'], f32).reshape(C, NB).mean(1)

    attn_rhs = np.zeros((65, 64), f32)
    attn_rhs[:64] = np.asarray(inp['attn_w'], f32).T
    attn_rhs[64] = np.asarray(inp['attn_b'], f32)
    out['attn_rhs'] = attn_rhs

    # channel duplicator 64->128 (for broadcasting attn over both halves)
    dupP = np.zeros((64, 128), f32)
    for c in range(64):
        dupP[c, c] = 1.0
        dupP[c, 64 + c] = 1.0
    out['dupP'] = dupP

    # resnet conv weights: BN-folded, permuted, padded to 384, 16x-scaled,
    # packed for DoubleRow as (q0,q1) pairs + q2 singles per (conv, m, k).
    res_pair = np.zeros((128, 4, 3, 3, 2, 128), f32)
    res_sing = np.zeros((128, 4, 3, 3, 128), f32)
    res_bias = np.zeros((4, 384), f32)
    ci = 0
    for blk in range(NBLK):
        for lyr in range(2):
            W = np.asarray(inp['res_w'], f32)[blk, lyr]
            g = np.asarray(inp['res_bn_g'], f32)[blk, lyr]
            b = np.asarray(inp['res_bn_b'], f32)[blk, lyr]
            m_ = np.asarray(inp['res_bn_m'], f32)[blk, lyr]
            v = np.asarray(inp['res_bn_v'], f32)[blk, lyr]
            inv = g / np.sqrt(v + EPS)
            Wf = W * inv[:, None, None]
            bf = b - m_ * inv
            Wp = Wf[PERM][:, PERM]                   # (320out, 320in, 3)
            Wpad = np.zeros((384, 384, 3), f32)
            Wpad[:C5, :C5] = Wp
            res_bias[ci] = np.pad(bf[PERM], (0, 64))
            WT = Wpad.transpose(1, 0, 2)             # lhsT (in, out, k)
            for m in range(3):
                for k in range(3):
                    for q in range(2):
                        res_pair[:, ci, m, k, q] = \
                            WT[q * 128:(q + 1) * 128, m * 128:(m + 1) * 128, k]
                    res_sing[:, ci, m, k] = \
                        WT[256:384, m * 128:(m + 1) * 128, k]
            ci += 1
    out['res_pair'] = res_pair.astype(bf16np)
    out['res_sing'] = res_sing.astype(bf16np)

    # proj lhsT: (128, {pair2|sing}, H) fp8, 16x-scaled
    pw = np.asarray(inp['proj_w'], f32)[:, PERM]     # (H, 320)
    pw_pad = np.zeros((H, 384), f32)
    pw_pad[:, :C5] = pw
    pwT = pw_pad.T * SQ                               # (384, H)
    out['proj_pair'] = np.ascontiguousarray(
        pwT[:256].reshape(2, 128, H).transpose(1, 0, 2)).astype(bf16np)
    out['proj_sing'] = np.ascontiguousarray(pwT[256:]).astype(bf16np)

    rwkv_lhsT = np.zeros((L, 4, H, H), f32)
    for l in range(L):
        rwkv_lhsT[l, 0] = np.asarray(inp['wk'], f32)[l].T
        rwkv_lhsT[l, 1] = np.asarray(inp['wv'], f32)[l].T
        rwkv_lhsT[l, 2] = np.asarray(inp['wr'], f32)[l].T
        rwkv_lhsT[l, 3] = np.asarray(inp['wo'], f32)[l].T
    out['rwkv_lhsT'] = np.ascontiguousarray(
        rwkv_lhsT.transpose(2, 0, 1, 3)).astype(bf16np)

    # LN gain rows (replicated across partitions) for K=1 broadcast matmuls
    lng = np.zeros((128, 2 * L, 128), f32)
    for l in range(L):
        lng[:, 2 * l + 0, :] = np.asarray(inp['ln1g'], f32)[l][None, :]
        lng[:, 2 * l + 1, :] = np.asarray(inp['ln2g'], f32)[l][None, :]
    out['lngain'] = lng.astype(bf16np)

    w1 = np.asarray(inp['cls_w1'], f32)
    out['cls1_lhsT'] = np.ascontiguousarray(w1.T.reshape(H, 2, 128))
    w2 = np.asarray(inp['cls_w2'], f32)
    out['cls2_lhsT'] = np.ascontiguousarray(
        w2.T.reshape(2, 128, NCLS).transpose(1, 0, 2))

    cols = {}
    def vec(name, v):
        cols[name] = np.asarray(v, f32)
    def pad128(v):
        o = np.zeros(128, f32); o[:len(v)] = v; return o

    vec('A', pad128(A)); vec('E0', pad128(E0)); vec('E1', pad128(E1))
    vec('E2', pad128(E2)); vec('E3', pad128(E3)); vec('Bb', pad128(Bb))
    for m in range(3):
        vec(f'band_b16_{m}', SQ * bb_pad[m * 128:(m + 1) * 128])
    for c4 in range(4):
        for m in range(3):
            vec(f'res_b16_{c4}_{m}', SQ * res_bias[c4, m * 128:(m + 1) * 128])
    vec('proj_b', np.asarray(inp['proj_b'], f32))
    for l in range(L):
        for w, nm in enumerate(['tmk', 'tmv', 'tmr']):
            tm = np.asarray(inp[nm], f32)[l]
            vec(f'tm{l}_{w}', tm)
            vec(f'tm1_{l}_{w}', (1.0 - tm) / T)
        vec(f'ln1g_{l}', np.asarray(inp['ln1g'], f32)[l])
        vec(f'ln1b_{l}', np.asarray(inp['ln1b'], f32)[l])
        vec(f'ln2g_{l}', np.asarray(inp['ln2g'], f32)[l])
        vec(f'ln2b_{l}', np.asarray(inp['ln2b'], f32)[l])
    vec('cls_b1a', np.asarray(inp['cls_b1'], f32)[:128])
    vec('cls_b1b', np.asarray(inp['cls_b1'], f32)[128:])
    vec('cls_b2', pad128(np.asarray(inp['cls_b2'], f32)))
    vec('eps', np.full(128, EPS, f32))

    names = list(cols.keys())
    out['cvec'] = np.ascontiguousarray(np.stack([cols[n] for n in names], 1))
    out['cvec_idx'] = {n: i for i, n in enumerate(names)}
    out['ln_trivial'] = bool(
        np.allclose(np.asarray(inp['ln1g'], f32), 1.0)
        and np.allclose(np.asarray(inp['ln1b'], f32), 0.0)
        and np.allclose(np.asarray(inp['ln2g'], f32), 1.0)
        and np.allclose(np.asarray(inp['ln2b'], f32), 0.0))
    return out


# ---------------------------------------------------------------------------
# bass kernel builder
# ---------------------------------------------------------------------------

def _build_nc(nv, ln_trivial=False, dbg_keys=()):
    nc = bacc.Bacc(None, target_bir_lowering=False)

    d_x = nc.dram_tensor('x', [B, 64, TP + 1], BF16, kind='ExternalInput')
    d_cvec = nc.dram_tensor('cvec', [128, nv], F32, kind='ExternalInput')
    d_attn = nc.dram_tensor('attn_rhs', [65, 64], F32R, kind='ExternalInput')
    d_dup = nc.dram_tensor('dupP', [64, 128], F32R, kind='ExternalInput')
    d_band = nc.dram_tensor('band_lhsT', [128, 3, 3, 128], BF16,
                            kind='ExternalInput')
    d_rp = nc.dram_tensor('res_pair', [128, 4, 3, 3, 2, 128], BF16,
                          kind='ExternalInput')
    d_rs = nc.dram_tensor('res_sing', [128, 4, 3, 3, 128], BF16,
                          kind='ExternalInput')
    d_pp = nc.dram_tensor('proj_pair', [128, 2, H], BF16, kind='ExternalInput')
    d_ps = nc.dram_tensor('proj_sing', [128, H], BF16, kind='ExternalInput')
    d_rwkv = nc.dram_tensor('rwkv_lhsT', [128, L, 4, H], BF16,
                            kind='ExternalInput')
    d_lng = nc.dram_tensor('lngain', [128, 2 * L, 128], BF16,
                           kind='ExternalInput')
    d_cls1 = nc.dram_tensor('cls1_lhsT', [128, 2, 128], F32R,
                            kind='ExternalInput')
    d_cls2 = nc.dram_tensor('cls2_lhsT', [128, 2, NCLS], F32R,
                            kind='ExternalInput')
    d_out = nc.dram_tensor('out', [NCLS, B], F32, kind='ExternalOutput')

    with tile.TileContext(nc) as tc:
        _emit(nc, tc, d_x, d_cvec, d_attn, d_dup, d_band, d_rp, d_rs,
              d_pp, d_ps, d_rwkv, d_lng, d_cls1, d_cls2, d_out, nv,
              ln_trivial, dbg_keys)
    nc.finalize()
    return nc


def _emit(nc, tc, d_x, d_cvec, d_attn, d_dup, d_band, d_rp, d_rs,
          d_pp, d_ps, d_rwkv, d_lng, d_cls1, d_cls2, d_out, nv,
          ln_trivial=False, dbg_keys=()):
    from contextlib import ExitStack

    def cap(key, ap):
        if key in dbg_keys:
            dt = nc.dram_tensor(f'dbg_{key}', list(ap.shape),
                                ap.dtype, kind='ExternalOutput')
            nc.gpsimd.dma_start(out=dt[...], in_=ap)

    ctx = ExitStack()
    with ctx:
        consts = ctx.enter_context(tc.tile_pool(name='consts', bufs=1))
        xp = ctx.enter_context(tc.tile_pool(name='xp', bufs=3))
        fo = ctx.enter_context(tc.tile_pool(name='fo', bufs=5))
        hp = ctx.enter_context(tc.tile_pool(name='hp', bufs=13))
        stats = ctx.enter_context(tc.tile_pool(name='stats', bufs=2))
        small = ctx.enter_context(tc.tile_pool(name='small', bufs=1))
        tmp = ctx.enter_context(tc.tile_pool(name='tmpc', bufs=1))
        psum = ctx.enter_context(tc.tile_pool(name='psum', bufs=1,
                                              space='PSUM'))

        def hpt(name):
            return hp.tile([128, T + 1], BF16, tag='hp', name=name)

        # ---------------- constants -----------------
        cvec = consts.tile([128, nv], F32)
        nc.gpsimd.dma_start(out=cvec, in_=d_cvec[:, :])
        CV = {}

        def colap(name):
            return cvec[:, CV[name]:CV[name] + 1]

        idx = 0
        def reg(name):
            nonlocal idx
            CV[name] = idx; idx += 1
        for n in ['A', 'E0', 'E1', 'E2', 'E3', 'Bb']:
            reg(n)
        for m in range(3):
            reg(f'band_b16_{m}')
        for c4 in range(4):
            for m in range(3):
                reg(f'res_b16_{c4}_{m}')
        reg('proj_b')
        for l in range(L):
            for w in range(3):
                reg(f'tm{l}_{w}')
                reg(f'tm1_{l}_{w}')
            for n in [f'ln1g_{l}', f'ln1b_{l}', f'ln2g_{l}', f'ln2b_{l}']:
                reg(n)
        for n in ['cls_b1a', 'cls_b1b', 'cls_b2', 'eps']:
            reg(n)
        assert idx == nv, (idx, nv)

        # ones/(H) column for LN sums (bf16: 1/128 is exact)
        onesH = consts.tile([128, 1], BF16)
        nc.vector.memset(onesH, 1.0 / H)
        decay = consts.tile([128, T], F32)
        nc.vector.memset(decay, 0.9)
        # f32r tiles cannot be memset directly; synthesize via ACT
        ones_lf = consts.tile([128, 128], F32R)
        nc.scalar.activation(out=ones_lf, in_=decay[:, 0:128], func=AF.Copy,
                             bias=1.0, scale=0.0)

        attn_rhs = consts.tile([65, 64], F32R)
        nc.gpsimd.dma_start(out=attn_rhs, in_=d_attn[:, :])
        dupP = consts.tile([64, 128], F32R)
        nc.gpsimd.dma_start(out=dupP, in_=d_dup[:, :])
        w_band = consts.tile([128, 3, 3, 128], BF16)
        nc.gpsimd.dma_start(out=w_band, in_=d_band[...])
        w_rp = consts.tile([128, 4, 3, 3, 2, 128], BF16)
        nc.gpsimd.dma_start(out=w_rp, in_=d_rp[...])
        w_rs = consts.tile([128, 4, 3, 3, 128], BF16)
        nc.gpsimd.dma_start(out=w_rs, in_=d_rs[...])
        w_pp = consts.tile([128, 2, H], BF16)
        nc.gpsimd.dma_start(out=w_pp, in_=d_pp[...])
        w_psg = consts.tile([128, H], BF16)
        nc.gpsimd.dma_start(out=w_psg, in_=d_ps[...])
        w_rwkv = consts.tile([128, L, 4, H], BF16)
        nc.gpsimd.dma_start(out=w_rwkv, in_=d_rwkv[...])
        w_lng = consts.tile([128, 2 * L, 128], BF16)
        nc.gpsimd.dma_start(out=w_lng, in_=d_lng[...])
        w_cls1 = consts.tile([128, 2, 128], F32R)
        nc.gpsimd.dma_start(out=w_cls1, in_=d_cls1[...])
        w_cls2 = consts.tile([128, 2, NCLS], F32R)
        nc.gpsimd.dma_start(out=w_cls2, in_=d_cls2[...])

        pooledHf = consts.tile([128, B], F32R)

        for g in range(NGROUP):
            _emit_group(nc, g, d_x, xp, fo, hp, hpt, stats, small, tmp, psum,
                        consts, colap, w_band, attn_rhs, dupP, w_rp, w_rs,
                        w_pp, w_psg, w_rwkv, w_lng, onesH, ones_lf,
                        decay, pooledHf, ln_trivial, cap)

        # ---------------- head ------------------------------------
        hidT = small.tile([128, 2, B], F32R)
        for mt in range(2):
            pt = psum.tile([128, B], F32, tag='bd', bufs=1, name=f'clsp{mt}')
            nc.tensor.matmul(pt, w_cls1[:, mt, :], pooledHf)
            nc.scalar.activation(out=hidT[:, mt, :], in_=pt, func=AF.Relu,
                                 bias=colap('cls_b1a' if mt == 0 else
                                            'cls_b1b'), scale=1.0)
        out_ps = psum.tile([NCLS, B], F32, tag='bd', bufs=1, name='out_ps')
        for kt in range(2):
            nc.tensor.matmul(out_ps, w_cls2[:, kt, :], hidT[:, kt, :],
                             start=(kt == 0), stop=(kt == 1))
        out_sb = small.tile([NCLS, B], F32)
        nc.scalar.activation(out=out_sb, in_=out_ps, func=AF.Identity,
                             bias=colap('cls_b2')[0:NCLS], scale=1.0)
        nc.gpsimd.dma_start(out=d_out[:, :], in_=out_sb)


def _emit_group(nc, g, d_x, xp, fo, hp, hpt, stats, small, tmp, psum,
                consts, colap, w_band, attn_rhs, dupP, w_rp, w_rs,
                w_pp, w_psg, w_rwkv, w_lng, onesH, ones_lf, decay,
                pooledHf, ln_trivial, cap):
    # ---------------- load x (plus shifted copy), pooled stats --------
    # xt rows 0-63 = x[b]; rows 64-127 = x[b] shifted left one column so
    # tap pairs (k, k+1) contract in a single K=128 matmul.
    xt = [xp.tile([128, TP], BF16, tag='xt', name=f'x{g}_{b}')
          for b in range(BL)]
    for b in range(BL):
        nc.sync.dma_start(out=xt[b][0:64, 0:TP],
                          in_=d_x[g * BL + b, :, 0:TP])
        nc.sync.dma_start(out=xt[b][64:128, 0:TP],
                          in_=d_x[g * BL + b, :, 1:TP + 1])
    S_b = small.tile([64, BL], F32, tag='sb', name=f'sb{g}')
    for b in range(BL):
        nc.vector.tensor_reduce(out=S_b[:, b:b + 1],
                                in_=xt[b][0:64, 2:2 + T],
                                axis=mybir.AxisListType.X, op=ALU.add)
    if g == 0:
        cap('x0', xt[0][:, :])
        cap('S_b', S_b[:, :])

    # pooled (transposed) + softmax over the 64 channels
    pooledT = small.tile([65, BL], F32R, tag='pt', name=f'pt{g}')
    nc.scalar.activation(out=pooledT[64:65, :], in_=S_b[0:1, 0:BL],
                         func=AF.Copy, bias=1.0, scale=0.0)
    for b in range(BL):
        p = pooledT[0:64, b:b + 1]
        nc.vector.tensor_scalar(
            out=p, in0=S_b[:, b:b + 1], scalar1=colap('A')[0:64],
            scalar2=colap('Bb')[0:64], op0=ALU.mult, op1=ALU.add)
        for name, cc in [('E0', 2), ('E1', 3), ('E2', T), ('E3', T + 1)]:
            nc.vector.scalar_tensor_tensor(
                out=p, in0=xt[b][0:64, cc:cc + 1],
                scalar=colap(name)[0:64], in1=p,
                op0=ALU.mult, op1=ALU.add)
    att_ps = psum.tile([64, BL], F32, tag='bd', bufs=1, name=f'attp{g}')
    nc.tensor.matmul(att_ps, attn_rhs, pooledT)
    attE = small.tile([64, BL], F32R, tag='attE', name=f'attE{g}')
    nc.scalar.activation(out=attE, in_=att_ps, func=AF.Exp)
    sum_ps = psum.tile([1, BL], F32, tag='bd', bufs=1, name=f'sump{g}')
    nc.tensor.matmul(sum_ps, ones_lf[0:64, 0:1], attE)
    arec = small.tile([1, BL], F32R, tag='arec', name=f'arec{g}')
    with nc.allow_low_precision(reason='softmax denom in fp32r is fine'):
        nc.vector.reciprocal(out=arec, in_=sum_ps)
    bc_ps = psum.tile([64, BL], F32, tag='bd', bufs=1, name=f'bcp{g}')
    nc.tensor.matmul(bc_ps, ones_lf[0:1, 0:64], arec, tile_position=(0, 0))
    attT = small.tile([64, BL], F32R, tag='attT', name=f'attT{g}')
    nc.vector.tensor_tensor(out=attT, in0=attE, in1=bc_ps, op=ALU.mult)
    # duplicate to 128 rows: avec_all[o,b] = attT[o%64,b], then 16x scale
    av_ps = psum.tile([128, BL], F32, tag='bd', bufs=1, name=f'avp{g}')
    nc.tensor.matmul(av_ps, dupP, attT)
    avec16 = small.tile([128, BL], F32, tag='av16', name=f'av16{g}')
    nc.scalar.activation(out=avec16, in_=av_ps, func=AF.Copy, scale=SQ)
    # bxa16[m] = band_b16_m * avec (the 16x is in band_b16)
    avec1 = small.tile([128, BL], F32, tag='av1', name=f'av1{g}')
    nc.vector.tensor_scalar(out=avec1, in0=av_ps, scalar1=1.0, scalar2=None,
                            op0=ALU.mult)
    bxa = small.tile([128, 3, BL], F32, tag='bxa', name=f'bxa{g}')
    for m in range(3):
        nc.gpsimd.tensor_scalar(out=bxa[:, m, :], in0=avec1,
                                scalar1=colap(f'band_b16_{m}'), scalar2=None,
                                op0=ALU.mult)
    if g == 0:
        cap('pooledT', pooledT[:, :])
        cap('attT', attT[:, :])

    # ---------------- band conv on PE -> F (fp8 Q16) -------------------
    F = [fo.tile([128, 3, TF], BF16, tag='fo', name=f'F{g}_{b}')
         for b in range(BL)]
    O = [fo.tile([128, 3, TF], BF16, tag='fo', name=f'O{g}_{b}')
         for b in range(BL)]
    for b in range(BL):
        for m in range(3):
            for t in (F, O):
                nc.gpsimd.memset(t[b][:, m, 1:2], 0.0)
                nc.gpsimd.memset(t[b][:, m, 2050:2051], 0.0)
    for b in range(BL):
        for m in range(3):
            for n in range(NCH):
                pt = psum.tile([128, CH], F32, tag='bd', bufs=1,
                               name=f'bc{g}_{b}_{m}_{n}')
                for tg in range(3):
                    nc.tensor.matmul(
                        pt, w_band[:, m, tg, :],
                        xt[b][:, CH * n + 2 * tg: CH * n + 2 * tg + CH],
                        start=(tg == 0), stop=(tg == 2))
                nc.scalar.activation(
                    out=F[b][:, m, 2 + CH * n: 2 + CH * (n + 1)], in_=pt,
                    func=AF.Identity, bias=bxa[:, m, b:b + 1],
                    scale=avec16[:, b:b + 1])
    if g == 0:
        cap('F00_band', F[0][:, :, :])

    # ---------------- resnet: 4 convs, fp8 DoubleRow -------------------
    def conv(c4, IN, OUT, residual):
        weights = [(k, q) for k in range(3) for q in range(3)]
        for b in range(BL):
            for m in range(3):
                bias = colap(f'res_b16_{c4}_{m}')
                for half in range(2):
                    pair = (2 * half, 2 * half + 1)
                    pts = {n: psum.tile([128, CH], F32, tag='cv', bufs=3,
                                        name=f'cv{g}_{c4}_{b}_{m}_{n}')
                           for n in pair}
                    # one ldweights per (k,q), applied to both psum banks
                    for wi, (k, q) in enumerate(weights):
                        w = (w_rp[:, c4, m, k, q, :] if q < 2
                             else w_rs[:, c4, m, k, :])
                        for n in pair:
                            nc.tensor.matmul(
                                pts[n], w,
                                IN[b][:, q, 1 + CH * n + k: 1 + CH * n + k + CH],
                                start=(wi == 0), stop=(wi == 8))
                    for n in pair:
                        pt = pts[n]
                        dst = OUT[b][:, m, 2 + CH * n: 2 + CH * (n + 1)]
                        if not residual:
                            # psum = 16*conv_true; dst = relu(psum + 16*bias)
                            nc.scalar.activation(
                                out=dst, in_=pt, func=AF.Relu,
                                bias=bias, scale=1.0)
                        else:
                            # dst = relu(psum + 16*bias + residual), all Q16
                            t2 = tmp.tile([128, CH], BF16, tag='cv', bufs=5,
                                          name=f'cvu{g}_{c4}_{b}_{m}_{n}')
                            nc.vector.scalar_tensor_tensor(
                                out=t2, in0=pt, scalar=bias, in1=dst,
                                op0=ALU.add, op1=ALU.add)
                            nc.vector.tensor_scalar(
                                out=dst, in0=t2, scalar1=0.0,
                                scalar2=None, op0=ALU.max)

    if 'noconv' not in os.environ.get('KABL', ''):
        conv(0, F, O, residual=False)
        conv(1, O, F, residual=True)
        conv(2, F, O, residual=False)
        conv(3, O, F, residual=True)
    if g == 0:
        cap('F00_res', F[0][:, :, :])

    # ---------------- proj --------------------------------------------
    h = [hpt(f'h{g}_{b}') for b in range(BL)]
    sums = [small.tile([128, 1], F32, tag='hsum', bufs=10,
                       name=f'hsum{g}_{b}') for b in range(BL)]
    for b in range(BL):
        for n in range(NCH):
            pt = psum.tile([128, CH], F32, tag='cv', bufs=3, name=f'pj{g}_{b}_{n}')
            w0 = 2 + CH * n
            for q in range(2):
                nc.tensor.matmul(pt, w_pp[:, q, :], F[b][:, q, w0: w0 + CH],
                                 start=(q == 0), stop=False)
            nc.tensor.matmul(pt, w_psg[:, :], F[b][:, 2, w0: w0 + CH],
                             start=False, stop=True)
            nc.scalar.activation(out=h[b][:, CH * n:CH * (n + 1)], in_=pt,
                                 func=AF.Identity, bias=colap('proj_b'),
                                 scale=1.0 / (SQ * SQ))
        nc.vector.tensor_reduce(out=sums[b], in_=h[b][:, 0:T],
                                axis=mybir.AxisListType.X, op=ALU.add)
    if g == 0:
        cap('h0', h[0][:, 0:T])

    # ---------------- rwkv layers --------------------------------------
    nl = 0 if 'norwkv' in os.environ.get('KABL', '') else L
    for l in range(nl):
        h, sums = _rwkv_layer(nc, g, hp, hpt, small, tmp, psum, stats,
                              colap, w_rwkv, w_lng, onesH, ones_lf,
                              decay, h, sums, l, ln_trivial, cap)
        if g == 0:
            cap(f'hn{l}_0', h[0][:, 0:T])

    # ---------------- pooled over T ------------------------------------
    for b in range(BL):
        nc.gpsimd.tensor_scalar(out=pooledHf[:, g * BL + b: g * BL + b + 1],
                                in0=sums[b], scalar1=1.0 / T, scalar2=None,
                                op0=ALU.mult)


def _rwkv_layer(nc, g, hp, hpt, small, tmp, psum, stats, colap,
                w_rwkv, w_lng, onesH, ones_lf, decay, h, sums, l,
                ln_trivial, cap):
    pre = f'{g}_{l}'
    # k/v/r: mix chunks on the fly, matmul, activation; ss = max(sk,.5)*vv
    tmv1 = {}
    for b in range(BL):
        for w in range(3):
            tv = small.tile([128, 1], F32, tag='tmv1', bufs=14,
                            name=f'tmv1_{pre}_{b}_{w}')
            nc.gpsimd.tensor_tensor(out=tv, in0=sums[b],
                                    in1=colap(f'tm1_{l}_{w}'), op=ALU.mult)
            tmv1[(b, w)] = tv
    ss = [hpt(f'ss{pre}_{b}') for b in range(BL)]
    rr = [hpt(f'rr{pre}_{b}') for b in range(BL)]
    alpha = [hpt(f'al{pre}_{b}') for b in range(BL)]
    for b in range(BL):
        for n in range(NCH):
            hc = h[b][:, CH * n:CH * (n + 1)]
            ck = {}
            for w in range(3):
                xw = tmp.tile([128, CH], BF16, tag='kv', bufs=3,
                              name=f'xw{pre}_{b}_{n}_{w}')
                nc.vector.tensor_scalar(
                    out=xw, in0=hc, scalar1=colap(f'tm{l}_{w}'),
                    scalar2=tmv1[(b, w)], op0=ALU.mult, op1=ALU.add)
                pt = psum.tile([128, CH], F32, tag='kv', bufs=2,
                               name=f'kvr{pre}_{b}_{w}_{n}')
                nc.tensor.matmul(pt, w_rwkv[:, l, w, :], xw)
                if w == 2:
                    nc.scalar.activation(out=rr[b][:, CH * n:CH * (n + 1)],
                                         in_=pt, func=AF.Sigmoid)
                elif w == 0:
                    cw = tmp.tile([128, CH], BF16, tag='kv', bufs=3,
                                  name=f'cw{pre}_{b}_{n}_{w}')
                    nc.scalar.activation(out=cw, in_=pt, func=AF.Sigmoid)
                    ck[w] = cw
                else:
                    cw = tmp.tile([128, CH], BF16, tag='kv', bufs=3,
                                  name=f'cw{pre}_{b}_{n}_{w}')
                    nc.vector.tensor_scalar(out=cw, in0=pt, scalar1=0.0,
                                            scalar2=None, op0=ALU.max)
                    ck[w] = cw
            nc.vector.scalar_tensor_tensor(
                out=ss[b][:, CH * n:CH * (n + 1)], in0=ck[0], scalar=0.5,
                in1=ck[1], op0=ALU.max, op1=ALU.mult)
    for b in range(BL):
        nc.gpsimd.memset(alpha[b][:, 0:1], 0.0)
        nc.vector.tensor_tensor_scan(
            out=alpha[b][:, 1:T + 1], data0=decay, data1=ss[b][:, 0:T],
            initial=0.0, op0=ALU.mult, op1=ALU.add)
        # wkv = alpha_t + 0.1*alpha_{t-1}; rwkv = r * wkv (into ss, in place)
        nc.vector.scalar_tensor_tensor(
            out=ss[b][:, 0:T], in0=alpha[b][:, 0:T], scalar=0.1,
            in1=alpha[b][:, 1:T + 1], op0=ALU.mult, op1=ALU.add)
        nc.vector.tensor_tensor(out=ss[b][:, 0:T], in0=rr[b][:, 0:T],
                                in1=ss[b][:, 0:T], op=ALU.mult)
    if l == 0 and g == 0:
        cap('alpha00', alpha[0][:, 0:T + 1])
        cap('rwkv00', ss[0][:, 0:T])
    # y = h + wo @ rwkv
    y = [hpt(f'y{pre}_{b}') for b in range(BL)]
    for b in range(BL):
        for n in range(NCH):
            pt = psum.tile([128, CH], F32, tag='kv', bufs=2,
                           name=f'op{pre}_{b}_{n}')
            nc.tensor.matmul(pt, w_rwkv[:, l, 3, :],
                             ss[b][:, CH * n:CH * (n + 1)])
            nc.vector.tensor_tensor(out=y[b][:, CH * n:CH * (n + 1)], in0=pt,
                                    in1=h[b][:, CH * n:CH * (n + 1)],
                                    op=ALU.add)
    if l == 0 and g == 0:
        cap('y00', y[0][:, 0:T])
    hn = [hpt(f'hn{pre}_{b}') for b in range(BL)]
    nsums = [small.tile([128, 1], F32, tag='hsum', bufs=10,
                        name=f'ns{pre}_{b}') for b in range(BL)]
    _ln(nc, g, hp, hpt, small, tmp, psum, stats, colap, onesH, w_lng,
        y, y, 2 * l, f'ln1b_{l}', tagp=f'l{pre}a')
    yn = y
    if l == 0 and g == 0:
        cap('yn00', yn[0][:, 0:T])
    if ln_trivial:
        # ln2(yhat) == yhat*(1/sqrt(1+eps)) when g==1,b==0; the 5e-6
        # scale error is far below tolerance: hn = yn + relu(yn)
        for b in range(BL):
            nc.vector.scalar_tensor_tensor(
                out=hn[b][:, 0:T], in0=yn[b][:, 0:T], scalar=0.0,
                in1=yn[b][:, 0:T], op0=ALU.max, op1=ALU.add,
                accum_out=nsums[b])
        return hn, nsums
    ffp = [hpt(f'ffp{pre}_{b}') for b in range(BL)]
    _ln(nc, g, hp, hpt, small, tmp, psum, stats, colap, onesH, w_lng,
        yn, ffp, 2 * l + 1, f'ln2b_{l}', tagp=f'l{pre}b')
    for b in range(BL):
        nc.vector.scalar_tensor_tensor(
            out=hn[b][:, 0:T], in0=ffp[b][:, 0:T], scalar=0.0,
            in1=yn[b][:, 0:T], op0=ALU.max, op1=ALU.add, accum_out=nsums[b])
    return hn, nsums


def _ln(nc, g, hp, hpt, small, tmp, psum, stats, colap, onesH, w_lng,
        y, out, grow, bname, tagp):
    """LayerNorm over the partition axis for each (batch, t) column.
    Stats rows live at partition 32*b of (128, T) tiles."""
    stat_y = stats.tile([128, T], BF16, tag='stat_y', name=f'sty_{tagp}')
    stat_q = stats.tile([128, T], BF16, tag='stat_q', name=f'stq_{tagp}')
    for n in range(NCH):
        p1 = psum.tile([128, CH], F32, tag='st', bufs=1, name=f'st1_{tagp}_{n}')
        p2 = psum.tile([128, CH], F32, tag='st2', bufs=1, name=f'st2_{tagp}_{n}')
        for b in range(BL):
            sq = tmp.tile([128, CH], BF16, tag='ln', bufs=3, name=f'sq{tagp}_{b}_{n}')
            nc.vector.tensor_tensor(out=sq, in0=y[b][:, CH * n:CH * (n + 1)],
                                    in1=y[b][:, CH * n:CH * (n + 1)],
                                    op=ALU.mult)
            nc.tensor.matmul(p1[32 * b:32 * b + 1, :], onesH,
                             y[b][:, CH * n:CH * (n + 1)],
                             tile_position=(0, 32 * b))
            nc.tensor.matmul(p2[32 * b:32 * b + 1, :], onesH, sq,
                             tile_position=(0, 32 * b))
        # mu, e2 (psum already scaled by 1/H via onesH)
        nc.vector.tensor_scalar(out=stat_y[:, CH * n:CH * (n + 1)], in0=p1,
                                scalar1=1.0, scalar2=None, op0=ALU.mult)
        nc.vector.tensor_scalar(out=stat_q[:, CH * n:CH * (n + 1)], in0=p2,
                                scalar1=float(EPS), scalar2=None, op0=ALU.add)
    # var+eps = (e2+eps) - mu^2; inv = sqrt(1/(var+eps)); negq = -mu*inv
    # (inv overwrites stat_q; the fp32 scratch is chunked to save SBUF)
    for n in range(NCH):
        cs = slice(CH * n, CH * (n + 1))
        sv32 = stats.tile([128, CH], F32, tag='sv32', bufs=2,
                          name=f'sv32_{tagp}_{n}')
        nc.gpsimd.tensor_tensor(out=sv32, in0=stat_y[:, cs],
                                in1=stat_y[:, cs], op=ALU.mult)
        nc.gpsimd.tensor_tensor(out=sv32, in0=stat_q[:, cs], in1=sv32,
                                op=ALU.subtract)
        nc.vector.reciprocal_approx_fast(out=sv32, in_=sv32)
        nc.scalar.activation(out=stat_q[:, cs], in_=sv32, func=AF.Sqrt)
    nc.vector.scalar_tensor_tensor(out=stat_y, in0=stat_y, scalar=-1.0,
                                   in1=stat_q, op0=ALU.mult, op1=ALU.mult)
    inv, negq = stat_q, stat_y
    bcol = colap(bname)
    for b in range(BL):
        pb = hpt(f'bcP{tagp}_{b}')
        qb = hpt(f'bcQ{tagp}_{b}')
        for n in range(NCH):
            bp = psum.tile([128, CH], F32, tag='st', bufs=1,
                           name=f'bp_{tagp}_{b}_{n}')
            bq = psum.tile([128, CH], F32, tag='st2', bufs=1,
                           name=f'bq_{tagp}_{b}_{n}')
            # pb = g[h] * inv[t]; qb = g[h] * negq[t] + beta[h]
            nc.tensor.matmul(bp, w_lng[32 * b:32 * b + 1, grow, :],
                             inv[32 * b:32 * b + 1, CH * n:CH * (n + 1)],
                             tile_position=(32 * b, 0))
            nc.tensor.matmul(bq, w_lng[32 * b:32 * b + 1, grow, :],
                             negq[32 * b:32 * b + 1, CH * n:CH * (n + 1)],
                             tile_position=(32 * b, 0))
            nc.scalar.activation(out=pb[:, CH * n:CH * (n + 1)], in_=bp,
                                 func=AF.Copy)
            nc.vector.tensor_scalar(out=qb[:, CH * n:CH * (n + 1)], in0=bq,
                                    scalar1=bcol, scalar2=None, op0=ALU.add)
        for n in range(NCH):
            tl = tmp.tile([128, CH], BF16, tag='ln', bufs=3, name=f'tl{tagp}_{b}_{n}')
            nc.vector.tensor_tensor(out=tl, in0=y[b][:, CH * n:CH * (n + 1)],
                                    in1=pb[:, CH * n:CH * (n + 1)],
                                    op=ALU.mult)
            nc.vector.tensor_tensor(out=out[b][:, CH * n:CH * (n + 1)],
                                    in0=tl, in1=qb[:, CH * n:CH * (n + 1)],
                                    op=ALU.add)


# ---------------------------------------------------------------------------
# entry point
# ---------------------------------------------------------------------------

_CACHE = {}


def kernel(**inputs):
    prep = _prep_weights(inputs)
    nv = prep['cvec'].shape[1]
    key = ('nc', prep['ln_trivial'])
    if key not in _CACHE:
        _CACHE[key] = _build_nc(nv, ln_trivial=prep['ln_trivial'])
    nc = _CACHE[key]
    _CACHE['nc'] = nc

    x = np.asarray(inputs['x'], np.float32).astype(bf16np)
    xc = x.transpose(0, 2, 1)                       # (B, C, T)
    xs = np.zeros((B, 64, TP + 1), dtype=bf16np)
    xs[:, :, 2:2 + T] = xc
    in_map = {
        'x': np.ascontiguousarray(xs),
        'cvec': prep['cvec'],
        'attn_rhs': prep['attn_rhs'], 'dupP': prep['dupP'],
        'band_lhsT': prep['band_lhsT'],
        'res_pair': prep['res_pair'], 'res_sing': prep['res_sing'],
        'proj_pair': prep['proj_pair'], 'proj_sing': prep['proj_sing'],
        'rwkv_lhsT': prep['rwkv_lhsT'], 'lngain': prep['lngain'],
        'cls1_lhsT': prep['cls1_lhsT'], 'cls2_lhsT': prep['cls2_lhsT'],
    }
    _CACHE['in_maps'] = [in_map]
    res = run_bass_kernel_spmd(nc, [in_map], core_ids=[0])
    out = res.results[0]['out']                     # (NCLS, B)
    return np.ascontiguousarray(out.T).astype(np.float32)


def profile_exec_ns():
    if 'nc' not in _CACHE or 'in_maps' not in _CACHE:
        return None
    try:
        res = run_bass_kernel_spmd(_CACHE['nc'], _CACHE['in_maps'],
                                   core_ids=[0], trace=True)
    except Exception as e:
        print("profile unavailable:", e)
        return None
    if res.instructions_and_trace is not None:
        print("trace:", res.instructions_and_trace[1])
    return res.exec_time_ns


def bench_exec(n=8):
    """Steady-state timing of the compiled executable (device-resident
    inputs, jit built once). Returns (min_s, avg_s) per call."""
    import time
    import jax
    from jax.sharding import Mesh, PartitionSpec
    from jax.experimental.shard_map import shard_map
    from concourse import bass2jax as b2j
    from concourse import mybir

    nc = _CACHE['nc']; in_maps = _CACHE['in_maps']
    b2j.install_neuronx_cc_hook()
    partition_name = (nc.partition_id_tensor.name
                      if nc.partition_id_tensor else None)
    in_names, out_names, out_avals, zero_outs = [], [], [], []
    for alloc in nc.m.functions[0].allocations:
        if not isinstance(alloc, mybir.MemoryLocationSet):
            continue
        name = alloc.memorylocations[0].name
        if alloc.kind == 'ExternalInput':
            if name != partition_name:
                in_names.append(name)
        elif alloc.kind == 'ExternalOutput':
            sh = tuple(alloc.tensor_shape)
            dt = mybir.dt.np(alloc.dtype)
            out_avals.append(jax.core.ShapedArray(sh, dt))
            out_names.append(name)
            zero_outs.append(np.zeros(sh, dt))
    n_params = len(in_names)
    n_outs = len(out_avals)
    all_in_names = list(in_names) + list(out_names)
    if partition_name is not None:
        all_in_names.append(partition_name)

    def _body(*args):
        operands = list(args)
        if partition_name is not None:
            operands.append(b2j.partition_id_tensor())
        outs = b2j._bass_exec_p.bind(
            *operands, out_avals=tuple(out_avals),
            in_names=tuple(all_in_names),
            out_names=tuple(out_names), lowering_input_output_aliases=(),
            sim_require_finite=True, sim_require_nnan=True, nc=nc)
        return tuple(outs)

    devices = jax.devices()[:NCORE]
    mesh = Mesh(np.asarray(devices), ('core',))
    in_specs = (PartitionSpec('core'),) * (n_params + n_outs)
    out_specs = (PartitionSpec('core'),) * len(out_names)
    sharded = jax.jit(shard_map(_body, mesh=mesh, in_specs=in_specs,
                                out_specs=out_specs, check_rep=False),
                      keep_unused=True)
    concat_in = [np.concatenate([np.asarray(in_maps[c][nm])
                                 for c in range(NCORE)], axis=0)
                 for nm in in_names]
    concat_zeros = [np.zeros((NCORE * z.shape[0], *z.shape[1:]), z.dtype)
                    for z in zero_outs]
    args = [jax.device_put(a) for a in concat_in + concat_zeros]
    r = sharded(*args); jax.block_until_ready(r)   # warmup/compile
    def run_n(k):
        t0 = time.perf_counter()
        rs = [sharded(*args) for _ in range(k)]
        jax.block_until_ready(rs)
        return time.perf_counter() - t0
    run_n(2)
    t1 = min(run_n(1) for _ in range(3))
    tn = min(run_n(n) for _ in range(3))
    slope = (tn - t1) / (n - 1)
    return t1, slope



# revision 49
# speedup vs baseline: 1.0839x; 1.0839x over previous
"""Trainium2 Bass kernel for nn_EEGMI_RWKV_ResNet_Model — single-core version.

Why one core: the per-exec metric (pipelined dispatch slope) carries
~1.3 ms of client/axon dispatch overhead PER DEVICE, serialized, for any
multi-device round — an empty 8-core kernel measures ~7.7 ms/exec. A
single-device shard_map dispatch pipelines with ~zero marginal overhead,
so the slope equals true device time. We therefore run all 32 batches on
core 0 and minimize device time.

Device-time design (per group of 4 batches, 8 groups streamed):
  - band conv on PE (depthwise as sparse 64->128 matmuls), attention scale
    fused into the psum-drain ACT (bias/scale APs), writing fp8 "Q16"
    (16x-scaled) activations.
  - resnet convs as fp8e4m3 DoubleRow matmuls: F tiles are (128, 3q, TF)
    so the (q0,q1) K-tile pair is one [K,2,N] AP; weights prepacked to
    match. Weights/activations are 16x-scaled into fp8's normal range;
    the 1/16 is folded into the psum drain.
  - rwkv: bf16 matmuls; elementwise work spread across DVE (2x/4x modes),
    ACT, and Pool (gpsimd) engines; the wkv scan is tensor_tensor_scan.
  - LayerNorm over the partition axis: sums via ones(1/H) matmuls,
    inv = ACT Rsqrt, per-t scalars broadcast over partitions via K=1
    matmuls with the gain vector as lhsT.
"""
import os
import numpy as np
import ml_dtypes

import concourse.bass as bass
import concourse.bacc as bacc
import concourse.tile as tile
from concourse import mybir
from concourse.bass_utils import run_bass_kernel_spmd

EPS = 1e-5
B, T, C = 32, 2048, 64
NB, C5, H, L, NBLK, NCLS = 5, 320, 128, 3, 2, 4
NCORE = 1
NGROUP = 16
BL = 2          # batches per group
NCH = 4
CH = 512
TP = T + 4      # padded width for band conv input
TF = T + 4      # conv tensor plane width (data cols 2..2050)
SQ = 16.0       # fp8 "Q16" scale for conv weights/activations

PERM = np.array([(o % 64) * 5 + (o // 64) for o in range(C5)], dtype=np.int64)

F32 = mybir.dt.float32
F32R = mybir.dt.float32r
BF16 = mybir.dt.bfloat16
FP8 = mybir.dt.float8e4
AF = mybir.ActivationFunctionType
ALU = mybir.AluOpType
DR = mybir.MatmulPerfMode.DoubleRow
bf16np = ml_dtypes.bfloat16
fp8np = ml_dtypes.float8_e4m3


# ---------------------------------------------------------------------------
# host-side weight preprocessing (numpy only)
# ---------------------------------------------------------------------------

def _prep_weights(inp):
    f32 = np.float32
    out = {}

    # band conv lhsT: (128, 3m, 3tg, 128) bf16.  out channel o' = j*64 + c
    # (j band, c channel); m block covers j = 2m, 2m+1 (m=2: j=4 only).
    # Tap pairs (0,1), (2,3), (4,-) packed along K: the x tile holds x in
    # partitions 0-63 and x shifted left by one column in partitions 64-127,
    # so tap 2tg sits in rows 0-63 and tap 2tg+1 in rows 64-127.
    bw = np.asarray(inp['band_w'], f32)[:, 0, :]   # (C5, 5) original order
    band_lhsT = np.zeros((128, 3, 3, 128), f32)
    for c in range(64):
        for j in range(NB):
            m, half = divmod(j, 2)
            for k in range(5):
                band_lhsT[(k % 2) * 64 + c, m, k // 2, half * 64 + c] = \
                    bw[c * 5 + j, k]
    out['band_lhsT'] = band_lhsT.astype(bf16np)

    bb = np.asarray(inp['band_b'], f32)[PERM]      # (320,) new order
    bb_pad = np.zeros((384,), f32)
    bb_pad[:C5] = bb

    # pooled-attention coefficients (same trick as before: pooled mean of the
    # band output equals an affine function of per-channel x sums + edge
    # corrections).
    bw_raw = bw.reshape(C, NB, 5)
    denom = f32(1.0 / (NB * T))
    A = bw_raw.sum(axis=(1, 2)) * denom
    E0 = -(bw_raw[:, :, 3] + bw_raw[:, :, 4]).sum(1) * denom
    E1 = -(bw_raw[:, :, 4]).sum(1) * denom
    E2 = -(bw_raw[:, :, 0]).sum(1) * denom
    E3 = -(bw_raw[:, :, 0] + bw_raw[:, :, 1]).sum(1) * denom
    Bb = np.asarray(inp['band_b'], f32).reshape(C, NB).mean(1)

    attn_rhs = np.zeros((65, 64), f32)
    attn_rhs[:64] = np.asarray(inp['attn_w'], f32).T
    attn_rhs[64] = np.asarray(inp['attn_b'], f32)
    out['attn_rhs'] = attn_rhs

    # channel duplicator 64->128 (for broadcasting attn over both halves)
    dupP = np.zeros((64, 128), f32)
    for c in range(64):
        dupP[c, c] = 1.0
        dupP[c, 64 + c] = 1.0
    out['dupP'] = dupP

    # resnet conv weights: BN-folded, permuted, padded to 384, 16x-scaled.
    # q0/q1 K-blocks per tap in res_pair; the 64-channel q2 remainder is
    # packed as (tap0 rows 0-63, tap1 rows 64-127) in res_q2p -- the q2
    # plane of the activation tiles carries a column-shifted copy of its
    # real rows in partitions 64-127 -- plus a K=64 single for tap2.
    res_pair = np.zeros((128, 4, 3, 3, 2, 128), f32)
    res_q2p = np.zeros((128, 4, 3, 128), f32)
    res_q2s = np.zeros((64, 4, 3, 128), f32)
    res_bias = np.zeros((4, 384), f32)
    ci = 0
    for blk in range(NBLK):
        for lyr in range(2):
            W = np.asarray(inp['res_w'], f32)[blk, lyr]
            g = np.asarray(inp['res_bn_g'], f32)[blk, lyr]
            b = np.asarray(inp['res_bn_b'], f32)[blk, lyr]
            m_ = np.asarray(inp['res_bn_m'], f32)[blk, lyr]
            v = np.asarray(inp['res_bn_v'], f32)[blk, lyr]
            inv = g / np.sqrt(v + EPS)
            Wf = W * inv[:, None, None]
            bf = b - m_ * inv
            Wp = Wf[PERM][:, PERM]                   # (320out, 320in, 3)
            Wpad = np.zeros((384, 384, 3), f32)
            Wpad[:C5, :C5] = Wp
            res_bias[ci] = np.pad(bf[PERM], (0, 64))
            WT = Wpad.transpose(1, 0, 2) * SQ        # lhsT (in, out, k), 16x
            for m in range(3):
                mc = slice(m * 128, (m + 1) * 128)
                for k in range(3):
                    for q in range(2):
                        res_pair[:, ci, m, k, q] = \
                            WT[q * 128:(q + 1) * 128, mc, k]
                res_q2p[0:64, ci, m] = WT[256:320, mc, 0]
                res_q2p[64:128, ci, m] = WT[256:320, mc, 1]
                res_q2s[:, ci, m] = WT[256:320, mc, 2]
            ci += 1
    out['res_pair'] = res_pair.astype(bf16np)
    out['res_q2p'] = res_q2p.astype(bf16np)
    out['res_q2s'] = res_q2s.astype(bf16np)

    # proj lhsT: (128, {pair2|sing}, H) fp8, 16x-scaled
    pw = np.asarray(inp['proj_w'], f32)[:, PERM]     # (H, 320)
    pw_pad = np.zeros((H, 384), f32)
    pw_pad[:, :C5] = pw
    pwT = pw_pad.T * SQ                               # (384, H)
    out['proj_pair'] = np.ascontiguousarray(
        pwT[:256].reshape(2, 128, H).transpose(1, 0, 2)).astype(bf16np)
    out['proj_sing'] = np.ascontiguousarray(pwT[256:]).astype(bf16np)

    rwkv_lhsT = np.zeros((L, 4, H, H), f32)
    for l in range(L):
        rwkv_lhsT[l, 0] = np.asarray(inp['wk'], f32)[l].T
        rwkv_lhsT[l, 1] = np.asarray(inp['wv'], f32)[l].T
        rwkv_lhsT[l, 2] = np.asarray(inp['wr'], f32)[l].T
        rwkv_lhsT[l, 3] = np.asarray(inp['wo'], f32)[l].T
    out['rwkv_lhsT'] = np.ascontiguousarray(
        rwkv_lhsT.transpose(2, 0, 1, 3)).astype(bf16np)

    # LN gain rows (replicated across partitions) for K=1 broadcast matmuls
    lng = np.zeros((128, 2 * L, 128), f32)
    for l in range(L):
        lng[:, 2 * l + 0, :] = np.asarray(inp['ln1g'], f32)[l][None, :]
        lng[:, 2 * l + 1, :] = np.asarray(inp['ln2g'], f32)[l][None, :]
    out['lngain'] = lng.astype(bf16np)

    w1 = np.asarray(inp['cls_w1'], f32)
    out['cls1_lhsT'] = np.ascontiguousarray(w1.T.reshape(H, 2, 128))
    w2 = np.asarray(inp['cls_w2'], f32)
    out['cls2_lhsT'] = np.ascontiguousarray(
        w2.T.reshape(2, 128, NCLS).transpose(1, 0, 2))

    cols = {}
    def vec(name, v):
        cols[name] = np.asarray(v, f32)
    def pad128(v):
        o = np.zeros(128, f32); o[:len(v)] = v; return o

    vec('A', pad128(A)); vec('E0', pad128(E0)); vec('E1', pad128(E1))
    vec('E2', pad128(E2)); vec('E3', pad128(E3)); vec('Bb', pad128(Bb))
    for m in range(3):
        vec(f'band_b16_{m}', SQ * bb_pad[m * 128:(m + 1) * 128])
    for c4 in range(4):
        for m in range(3):
            vec(f'res_b16_{c4}_{m}', SQ * res_bias[c4, m * 128:(m + 1) * 128])
    vec('proj_b', np.asarray(inp['proj_b'], f32))
    for l in range(L):
        for w, nm in enumerate(['tmk', 'tmv', 'tmr']):
            tm = np.asarray(inp[nm], f32)[l]
            vec(f'tm{l}_{w}', tm)
            vec(f'tm1_{l}_{w}', (1.0 - tm) / T)
        vec(f'ln1g_{l}', np.asarray(inp['ln1g'], f32)[l])
        vec(f'ln1b_{l}', np.asarray(inp['ln1b'], f32)[l])
        vec(f'ln2g_{l}', np.asarray(inp['ln2g'], f32)[l])
        vec(f'ln2b_{l}', np.asarray(inp['ln2b'], f32)[l])
    vec('cls_b1a', np.asarray(inp['cls_b1'], f32)[:128])
    vec('cls_b1b', np.asarray(inp['cls_b1'], f32)[128:])
    vec('cls_b2', pad128(np.asarray(inp['cls_b2'], f32)))
    vec('eps', np.full(128, EPS, f32))

    names = list(cols.keys())
    out['cvec'] = np.ascontiguousarray(np.stack([cols[n] for n in names], 1))
    out['cvec_idx'] = {n: i for i, n in enumerate(names)}
    out['ln_trivial'] = bool(
        np.allclose(np.asarray(inp['ln1g'], f32), 1.0)
        and np.allclose(np.asarray(inp['ln1b'], f32), 0.0)
        and np.allclose(np.asarray(inp['ln2g'], f32), 1.0)
        and np.allclose(np.asarray(inp['ln2b'], f32), 0.0))
    return out


# ---------------------------------------------------------------------------
# bass kernel builder
# ---------------------------------------------------------------------------

def _build_nc(nv, ln_trivial=False, dbg_keys=()):
    nc = bacc.Bacc(None, target_bir_lowering=False)

    d_x = nc.dram_tensor('x', [B, 64, TP + 1], BF16, kind='ExternalInput')
    d_cvec = nc.dram_tensor('cvec', [128, nv], F32, kind='ExternalInput')
    d_attn = nc.dram_tensor('attn_rhs', [65, 64], F32R, kind='ExternalInput')
    d_dup = nc.dram_tensor('dupP', [64, 128], F32R, kind='ExternalInput')
    d_band = nc.dram_tensor('band_lhsT', [128, 3, 3, 128], BF16,
                            kind='ExternalInput')
    d_rp = nc.dram_tensor('res_pair', [128, 4, 3, 3, 2, 128], BF16,
                          kind='ExternalInput')
    d_q2p = nc.dram_tensor('res_q2p', [128, 4, 3, 128], BF16,
                           kind='ExternalInput')
    d_q2s = nc.dram_tensor('res_q2s', [64, 4, 3, 128], BF16,
                           kind='ExternalInput')
    d_pp = nc.dram_tensor('proj_pair', [128, 2, H], BF16, kind='ExternalInput')
    d_ps = nc.dram_tensor('proj_sing', [128, H], BF16, kind='ExternalInput')
    d_rwkv = nc.dram_tensor('rwkv_lhsT', [128, L, 4, H], BF16,
                            kind='ExternalInput')
    d_lng = nc.dram_tensor('lngain', [128, 2 * L, 128], BF16,
                           kind='ExternalInput')
    d_cls1 = nc.dram_tensor('cls1_lhsT', [128, 2, 128], F32R,
                            kind='ExternalInput')
    d_cls2 = nc.dram_tensor('cls2_lhsT', [128, 2, NCLS], F32R,
                            kind='ExternalInput')
    d_out = nc.dram_tensor('out', [NCLS, B], F32, kind='ExternalOutput')

    with tile.TileContext(nc) as tc:
        _emit(nc, tc, d_x, d_cvec, d_attn, d_dup, d_band, d_rp, d_q2p, d_q2s,
              d_pp, d_ps, d_rwkv, d_lng, d_cls1, d_cls2, d_out, nv,
              ln_trivial, dbg_keys)
    nc.finalize()
    return nc


def _emit(nc, tc, d_x, d_cvec, d_attn, d_dup, d_band, d_rp, d_q2p, d_q2s,
          d_pp, d_ps, d_rwkv, d_lng, d_cls1, d_cls2, d_out, nv,
          ln_trivial=False, dbg_keys=()):
    from contextlib import ExitStack

    def cap(key, ap):
        if key in dbg_keys:
            dt = nc.dram_tensor(f'dbg_{key}', list(ap.shape),
                                ap.dtype, kind='ExternalOutput')
            nc.gpsimd.dma_start(out=dt[...], in_=ap)

    ctx = ExitStack()
    with ctx:
        consts = ctx.enter_context(tc.tile_pool(name='consts', bufs=1))
        xp = ctx.enter_context(tc.tile_pool(name='xp', bufs=3))
        fo = ctx.enter_context(tc.tile_pool(name='fo', bufs=5))
        hp = ctx.enter_context(tc.tile_pool(name='hp', bufs=13))
        stats = ctx.enter_context(tc.tile_pool(name='stats', bufs=2))
        small = ctx.enter_context(tc.tile_pool(name='small', bufs=1))
        tmp = ctx.enter_context(tc.tile_pool(name='tmpc', bufs=1))
        psum = ctx.enter_context(tc.tile_pool(name='psum', bufs=1,
                                              space='PSUM'))

        def hpt(name):
            return hp.tile([128, T + 1], BF16, tag='hp', name=name)

        # ---------------- constants -----------------
        cvec = consts.tile([128, nv], F32)
        nc.gpsimd.dma_start(out=cvec, in_=d_cvec[:, :])
        CV = {}

        def colap(name):
            return cvec[:, CV[name]:CV[name] + 1]

        idx = 0
        def reg(name):
            nonlocal idx
            CV[name] = idx; idx += 1
        for n in ['A', 'E0', 'E1', 'E2', 'E3', 'Bb']:
            reg(n)
        for m in range(3):
            reg(f'band_b16_{m}')
        for c4 in range(4):
            for m in range(3):
                reg(f'res_b16_{c4}_{m}')
        reg('proj_b')
        for l in range(L):
            for w in range(3):
                reg(f'tm{l}_{w}')
                reg(f'tm1_{l}_{w}')
            for n in [f'ln1g_{l}', f'ln1b_{l}', f'ln2g_{l}', f'ln2b_{l}']:
                reg(n)
        for n in ['cls_b1a', 'cls_b1b', 'cls_b2', 'eps']:
            reg(n)
        assert idx == nv, (idx, nv)

        # ones/(H) column for LN sums (bf16: 1/128 is exact)
        onesH = consts.tile([128, 1], BF16)
        nc.vector.memset(onesH, 1.0 / H)
        decay = consts.tile([128, T], F32)
        nc.vector.memset(decay, 0.9)
        # f32r tiles cannot be memset directly; synthesize via ACT
        ones_lf = consts.tile([128, 128], F32R)
        nc.scalar.activation(out=ones_lf, in_=decay[:, 0:128], func=AF.Copy,
                             bias=1.0, scale=0.0)

        attn_rhs = consts.tile([65, 64], F32R)
        nc.gpsimd.dma_start(out=attn_rhs, in_=d_attn[:, :])
        dupP = consts.tile([64, 128], F32R)
        nc.gpsimd.dma_start(out=dupP, in_=d_dup[:, :])
        w_band = consts.tile([128, 3, 3, 128], BF16)
        nc.gpsimd.dma_start(out=w_band, in_=d_band[...])
        w_rp = consts.tile([128, 4, 3, 3, 2, 128], BF16)
        nc.gpsimd.dma_start(out=w_rp, in_=d_rp[...])
        w_q2p = consts.tile([128, 4, 3, 128], BF16)
        nc.gpsimd.dma_start(out=w_q2p, in_=d_q2p[...])
        w_q2s = consts.tile([64, 4, 3, 128], BF16)
        nc.gpsimd.dma_start(out=w_q2s, in_=d_q2s[...])
        w_pp = consts.tile([128, 2, H], BF16)
        nc.gpsimd.dma_start(out=w_pp, in_=d_pp[...])
        w_psg = consts.tile([128, H], BF16)
        nc.gpsimd.dma_start(out=w_psg, in_=d_ps[...])
        w_rwkv = consts.tile([128, L, 4, H], BF16)
        nc.gpsimd.dma_start(out=w_rwkv, in_=d_rwkv[...])
        w_lng = consts.tile([128, 2 * L, 128], BF16)
        nc.gpsimd.dma_start(out=w_lng, in_=d_lng[...])
        w_cls1 = consts.tile([128, 2, 128], F32R)
        nc.gpsimd.dma_start(out=w_cls1, in_=d_cls1[...])
        w_cls2 = consts.tile([128, 2, NCLS], F32R)
        nc.gpsimd.dma_start(out=w_cls2, in_=d_cls2[...])

        pooledHf = consts.tile([128, B], F32R)

        for g in range(NGROUP):
            _emit_group(nc, g, d_x, xp, fo, hp, hpt, stats, small, tmp, psum,
                        consts, colap, w_band, attn_rhs, dupP, w_rp,
                        w_q2p, w_q2s, w_pp, w_psg, w_rwkv, w_lng, onesH,
                        ones_lf, decay, pooledHf, ln_trivial, cap)

        # ---------------- head ------------------------------------
        hidT = small.tile([128, 2, B], F32R)
        for mt in range(2):
            pt = psum.tile([128, B], F32, tag='bd', bufs=1, name=f'clsp{mt}')
            nc.tensor.matmul(pt, w_cls1[:, mt, :], pooledHf)
            nc.scalar.activation(out=hidT[:, mt, :], in_=pt, func=AF.Relu,
                                 bias=colap('cls_b1a' if mt == 0 else
                                            'cls_b1b'), scale=1.0)
        out_ps = psum.tile([NCLS, B], F32, tag='bd', bufs=1, name='out_ps')
        for kt in range(2):
            nc.tensor.matmul(out_ps, w_cls2[:, kt, :], hidT[:, kt, :],
                             start=(kt == 0), stop=(kt == 1))
        out_sb = small.tile([NCLS, B], F32)
        nc.scalar.activation(out=out_sb, in_=out_ps, func=AF.Identity,
                             bias=colap('cls_b2')[0:NCLS], scale=1.0)
        nc.gpsimd.dma_start(out=d_out[:, :], in_=out_sb)


def _emit_group(nc, g, d_x, xp, fo, hp, hpt, stats, small, tmp, psum,
                consts, colap, w_band, attn_rhs, dupP, w_rp,
                w_q2p, w_q2s, w_pp, w_psg, w_rwkv, w_lng, onesH,
                ones_lf, decay, pooledHf, ln_trivial, cap):
    # ---------------- load x (plus shifted copy), pooled stats --------
    # xt rows 0-63 = x[b]; rows 64-127 = x[b] shifted left one column so
    # tap pairs (k, k+1) contract in a single K=128 matmul.
    xt = [xp.tile([128, TP], BF16, tag='xt', name=f'x{g}_{b}')
          for b in range(BL)]
    for b in range(BL):
        nc.sync.dma_start(out=xt[b][0:64, 0:TP],
                          in_=d_x[g * BL + b, :, 0:TP])
        nc.sync.dma_start(out=xt[b][64:128, 0:TP],
                          in_=d_x[g * BL + b, :, 1:TP + 1])
    S_b = small.tile([64, BL], F32, tag='sb', name=f'sb{g}')
    for b in range(BL):
        nc.vector.tensor_reduce(out=S_b[:, b:b + 1],
                                in_=xt[b][0:64, 2:2 + T],
                                axis=mybir.AxisListType.X, op=ALU.add)
    if g == 0:
        cap('x0', xt[0][:, :])
        cap('S_b', S_b[:, :])

    # pooled (transposed) + softmax over the 64 channels
    pooledT = small.tile([65, BL], F32R, tag='pt', name=f'pt{g}')
    nc.scalar.activation(out=pooledT[64:65, :], in_=S_b[0:1, 0:BL],
                         func=AF.Copy, bias=1.0, scale=0.0)
    for b in range(BL):
        p = pooledT[0:64, b:b + 1]
        nc.vector.tensor_scalar(
            out=p, in0=S_b[:, b:b + 1], scalar1=colap('A')[0:64],
            scalar2=colap('Bb')[0:64], op0=ALU.mult, op1=ALU.add)
        for name, cc in [('E0', 2), ('E1', 3), ('E2', T), ('E3', T + 1)]:
            nc.vector.scalar_tensor_tensor(
                out=p, in0=xt[b][0:64, cc:cc + 1],
                scalar=colap(name)[0:64], in1=p,
                op0=ALU.mult, op1=ALU.add)
    att_ps = psum.tile([64, BL], F32, tag='bd', bufs=1, name=f'attp{g}')
    nc.tensor.matmul(att_ps, attn_rhs, pooledT)
    attE = small.tile([64, BL], F32R, tag='attE', name=f'attE{g}')
    nc.scalar.activation(out=attE, in_=att_ps, func=AF.Exp)
    sum_ps = psum.tile([1, BL], F32, tag='bd', bufs=1, name=f'sump{g}')
    nc.tensor.matmul(sum_ps, ones_lf[0:64, 0:1], attE)
    arec = small.tile([1, BL], F32R, tag='arec', name=f'arec{g}')
    with nc.allow_low_precision(reason='softmax denom in fp32r is fine'):
        nc.vector.reciprocal(out=arec, in_=sum_ps)
    bc_ps = psum.tile([64, BL], F32, tag='bd', bufs=1, name=f'bcp{g}')
    nc.tensor.matmul(bc_ps, ones_lf[0:1, 0:64], arec, tile_position=(0, 0))
    attT = small.tile([64, BL], F32R, tag='attT', name=f'attT{g}')
    nc.vector.tensor_tensor(out=attT, in0=attE, in1=bc_ps, op=ALU.mult)
    # duplicate to 128 rows: avec_all[o,b] = attT[o%64,b], then 16x scale
    av_ps = psum.tile([128, BL], F32, tag='bd', bufs=1, name=f'avp{g}')
    nc.tensor.matmul(av_ps, dupP, attT)
    avec16 = small.tile([128, BL], F32, tag='av16', name=f'av16{g}')
    nc.scalar.activation(out=avec16, in_=av_ps, func=AF.Copy, scale=SQ)
    # bxa16[m] = band_b16_m * avec (the 16x is in band_b16)
    avec1 = small.tile([128, BL], F32, tag='av1', name=f'av1{g}')
    nc.vector.tensor_scalar(out=avec1, in0=av_ps, scalar1=1.0, scalar2=None,
                            op0=ALU.mult)
    bxa = small.tile([128, 3, BL], F32, tag='bxa', name=f'bxa{g}')
    for m in range(3):
        nc.gpsimd.tensor_scalar(out=bxa[:, m, :], in0=avec1,
                                scalar1=colap(f'band_b16_{m}'), scalar2=None,
                                op0=ALU.mult)
    if g == 0:
        cap('pooledT', pooledT[:, :])
        cap('attT', attT[:, :])

    # ---------------- band conv on PE -> F (fp8 Q16) -------------------
    F = [fo.tile([128, 3, TF], BF16, tag='fo', name=f'F{g}_{b}')
         for b in range(BL)]
    O = [fo.tile([128, 3, TF], BF16, tag='fo', name=f'O{g}_{b}')
         for b in range(BL)]
    for b in range(BL):
        for m in range(3):
            for t in (F, O):
                nc.gpsimd.memset(t[b][:, m, 1:2], 0.0)
                nc.gpsimd.memset(t[b][:, m, 2050:2051], 0.0)
    for b in range(BL):
        for m in range(3):
            for n in range(NCH):
                pt = psum.tile([128, CH], F32, tag='bd', bufs=1,
                               name=f'bc{g}_{b}_{m}_{n}')
                for tg in range(3):
                    nc.tensor.matmul(
                        pt, w_band[:, m, tg, :],
                        xt[b][:, CH * n + 2 * tg: CH * n + 2 * tg + CH],
                        start=(tg == 0), stop=(tg == 2))
                nc.scalar.activation(
                    out=F[b][:, m, 2 + CH * n: 2 + CH * (n + 1)], in_=pt,
                    func=AF.Identity, bias=bxa[:, m, b:b + 1],
                    scale=avec16[:, b:b + 1])
    if g == 0:
        cap('F00_band', F[0][:, :, :])

    # ---------------- resnet: 4 convs, bf16 ----------------------------
    # rshift(X): X's q2 plane gets a column-shifted copy of its real rows
    # in partitions 64-127, so the q2 remainder taps (0,1) contract as one
    # K=128 matmul (weights res_q2p) and tap 2 as a K=64 single.
    def rshift(X):
        for b in range(BL):
            nc.gpsimd.dma_start(out=X[b][64:128, 2, 0:2050],
                                in_=X[b][0:64, 2, 1:2051])

    def conv(c4, IN, OUT, residual):
        weights = [(k, q) for k in range(3) for q in range(2)]
        for b in range(BL):
            for m in range(3):
                bias = colap(f'res_b16_{c4}_{m}')
                for half in range(2):
                    pair = (2 * half, 2 * half + 1)
                    pts = {n: psum.tile([128, CH], F32, tag='cv', bufs=3,
                                        name=f'cv{g}_{c4}_{b}_{m}_{n}')
                           for n in pair}
                    # one ldweights per weight, applied to both psum banks
                    for wi, (k, q) in enumerate(weights):
                        w = w_rp[:, c4, m, k, q, :]
                        for n in pair:
                            nc.tensor.matmul(
                                pts[n], w,
                                IN[b][:, q, 1 + CH * n + k: 1 + CH * n + k + CH],
                                start=(wi == 0), stop=False)
                    for n in pair:
                        nc.tensor.matmul(
                            pts[n], w_q2p[:, c4, m, :],
                            IN[b][:, 2, 1 + CH * n: 1 + CH * n + CH],
                            start=False, stop=False)
                    for n in pair:
                        nc.tensor.matmul(
                            pts[n], w_q2s[:, c4, m, :],
                            IN[b][0:64, 2, 3 + CH * n: 3 + CH * n + CH],
                            start=False, stop=True)
                    for n in pair:
                        pt = pts[n]
                        dst = OUT[b][:, m, 2 + CH * n: 2 + CH * (n + 1)]
                        if not residual:
                            nc.scalar.activation(
                                out=dst, in_=pt, func=AF.Relu,
                                bias=bias, scale=1.0 / SQ)
                        else:
                            t1 = tmp.tile([128, CH], BF16, tag='cv', bufs=5,
                                          name=f'cvt{g}_{c4}_{b}_{m}_{n}')
                            nc.scalar.activation(
                                out=t1, in_=pt, func=AF.Identity,
                                bias=bias, scale=1.0 / SQ)
                            t2 = tmp.tile([128, CH], BF16, tag='cv', bufs=5,
                                          name=f'cvu{g}_{c4}_{b}_{m}_{n}')
                            nc.gpsimd.tensor_tensor(out=t2, in0=t1, in1=dst,
                                                    op=ALU.add)
                            nc.vector.tensor_scalar(
                                out=dst, in0=t2, scalar1=0.0,
                                scalar2=None, op0=ALU.max)

    if 'noconv' not in os.environ.get('KABL', ''):
        rshift(F)
        conv(0, F, O, residual=False)
        rshift(O)
        conv(1, O, F, residual=True)
        rshift(F)
        conv(2, F, O, residual=False)
        rshift(O)
        conv(3, O, F, residual=True)
    if g == 0:
        cap('F00_res', F[0][:, :, :])

    # ---------------- proj --------------------------------------------
    h = [hpt(f'h{g}_{b}') for b in range(BL)]
    sums = [small.tile([128, 1], F32, tag='hsum', bufs=10,
                       name=f'hsum{g}_{b}') for b in range(BL)]
    for b in range(BL):
        for n in range(NCH):
            pt = psum.tile([128, CH], F32, tag='cv', bufs=3, name=f'pj{g}_{b}_{n}')
            w0 = 2 + CH * n
            for q in range(2):
                nc.tensor.matmul(pt, w_pp[:, q, :], F[b][:, q, w0: w0 + CH],
                                 start=(q == 0), stop=False)
            nc.tensor.matmul(pt, w_psg[:, :], F[b][:, 2, w0: w0 + CH],
                             start=False, stop=True)
            nc.scalar.activation(out=h[b][:, CH * n:CH * (n + 1)], in_=pt,
                                 func=AF.Identity, bias=colap('proj_b'),
                                 scale=1.0 / (SQ * SQ))
        nc.vector.tensor_reduce(out=sums[b], in_=h[b][:, 0:T],
                                axis=mybir.AxisListType.X, op=ALU.add)
    if g == 0:
        cap('h0', h[0][:, 0:T])

    # ---------------- rwkv layers --------------------------------------
    nl = 0 if 'norwkv' in os.environ.get('KABL', '') else L
    for l in range(nl):
        h, sums = _rwkv_layer(nc, g, hp, hpt, small, tmp, psum, stats,
                              colap, w_rwkv, w_lng, onesH, ones_lf,
                              decay, h, sums, l, ln_trivial, cap)
        if g == 0:
            cap(f'hn{l}_0', h[0][:, 0:T])

    # ---------------- pooled over T ------------------------------------
    for b in range(BL):
        nc.gpsimd.tensor_scalar(out=pooledHf[:, g * BL + b: g * BL + b + 1],
                                in0=sums[b], scalar1=1.0 / T, scalar2=None,
                                op0=ALU.mult)


def _rwkv_layer(nc, g, hp, hpt, small, tmp, psum, stats, colap,
                w_rwkv, w_lng, onesH, ones_lf, decay, h, sums, l,
                ln_trivial, cap):
    pre = f'{g}_{l}'
    # k/v/r: mix chunks on the fly, matmul, activation; ss = max(sk,.5)*vv
    tmv1 = {}
    for b in range(BL):
        for w in range(3):
            tv = small.tile([128, 1], F32, tag='tmv1', bufs=14,
                            name=f'tmv1_{pre}_{b}_{w}')
            nc.gpsimd.tensor_tensor(out=tv, in0=sums[b],
                                    in1=colap(f'tm1_{l}_{w}'), op=ALU.mult)
            tmv1[(b, w)] = tv
    ss = [hpt(f'ss{pre}_{b}') for b in range(BL)]
    rr = [hpt(f'rr{pre}_{b}') for b in range(BL)]
    alpha = [hpt(f'al{pre}_{b}') for b in range(BL)]
    for b in range(BL):
        for n in range(NCH):
            hc = h[b][:, CH * n:CH * (n + 1)]
            ck = {}
            for w in range(3):
                xw = tmp.tile([128, CH], BF16, tag='kv', bufs=3,
                              name=f'xw{pre}_{b}_{n}_{w}')
                nc.vector.tensor_scalar(
                    out=xw, in0=hc, scalar1=colap(f'tm{l}_{w}'),
                    scalar2=tmv1[(b, w)], op0=ALU.mult, op1=ALU.add)
                pt = psum.tile([128, CH], F32, tag='kv', bufs=2,
                               name=f'kvr{pre}_{b}_{w}_{n}')
                nc.tensor.matmul(pt, w_rwkv[:, l, w, :], xw)
                if w == 2:
                    nc.scalar.activation(out=rr[b][:, CH * n:CH * (n + 1)],
                                         in_=pt, func=AF.Sigmoid)
                elif w == 0:
                    cw = tmp.tile([128, CH], BF16, tag='kv', bufs=3,
                                  name=f'cw{pre}_{b}_{n}_{w}')
                    nc.scalar.activation(out=cw, in_=pt, func=AF.Sigmoid)
                    ck[w] = cw
                else:
                    cw = tmp.tile([128, CH], BF16, tag='kv', bufs=3,
                                  name=f'cw{pre}_{b}_{n}_{w}')
                    nc.vector.tensor_scalar(out=cw, in0=pt, scalar1=0.0,
                                            scalar2=None, op0=ALU.max)
                    ck[w] = cw
            nc.vector.scalar_tensor_tensor(
                out=ss[b][:, CH * n:CH * (n + 1)], in0=ck[0], scalar=0.5,
                in1=ck[1], op0=ALU.max, op1=ALU.mult)
    for b in range(BL):
        nc.gpsimd.memset(alpha[b][:, 0:1], 0.0)
        nc.vector.tensor_tensor_scan(
            out=alpha[b][:, 1:T + 1], data0=decay, data1=ss[b][:, 0:T],
            initial=0.0, op0=ALU.mult, op1=ALU.add)
        # wkv = alpha_t + 0.1*alpha_{t-1}; rwkv = r * wkv (into ss, in place)
        nc.vector.scalar_tensor_tensor(
            out=ss[b][:, 0:T], in0=alpha[b][:, 0:T], scalar=0.1,
            in1=alpha[b][:, 1:T + 1], op0=ALU.mult, op1=ALU.add)
        nc.vector.tensor_tensor(out=ss[b][:, 0:T], in0=rr[b][:, 0:T],
                                in1=ss[b][:, 0:T], op=ALU.mult)
    if l == 0 and g == 0:
        cap('alpha00', alpha[0][:, 0:T + 1])
        cap('rwkv00', ss[0][:, 0:T])
    # y = h + wo @ rwkv
    y = [hpt(f'y{pre}_{b}') for b in range(BL)]
    for b in range(BL):
        for n in range(NCH):
            pt = psum.tile([128, CH], F32, tag='kv', bufs=2,
                           name=f'op{pre}_{b}_{n}')
            nc.tensor.matmul(pt, w_rwkv[:, l, 3, :],
                             ss[b][:, CH * n:CH * (n + 1)])
            nc.vector.tensor_tensor(out=y[b][:, CH * n:CH * (n + 1)], in0=pt,
                                    in1=h[b][:, CH * n:CH * (n + 1)],
                                    op=ALU.add)
    if l == 0 and g == 0:
        cap('y00', y[0][:, 0:T])
    hn = [hpt(f'hn{pre}_{b}') for b in range(BL)]
    nsums = [small.tile([128, 1], F32, tag='hsum', bufs=10,
                        name=f'ns{pre}_{b}') for b in range(BL)]
    _ln(nc, g, hp, hpt, small, tmp, psum, stats, colap, onesH, w_lng,
        y, y, 2 * l, f'ln1b_{l}', tagp=f'l{pre}a')
    yn = y
    if l == 0 and g == 0:
        cap('yn00', yn[0][:, 0:T])
    if ln_trivial:
        # ln2(yhat) == yhat*(1/sqrt(1+eps)) when g==1,b==0; the 5e-6
        # scale error is far below tolerance: hn = yn + relu(yn)
        for b in range(BL):
            nc.vector.scalar_tensor_tensor(
                out=hn[b][:, 0:T], in0=yn[b][:, 0:T], scalar=0.0,
                in1=yn[b][:, 0:T], op0=ALU.max, op1=ALU.add,
                accum_out=nsums[b])
        return hn, nsums
    ffp = [hpt(f'ffp{pre}_{b}') for b in range(BL)]
    _ln(nc, g, hp, hpt, small, tmp, psum, stats, colap, onesH, w_lng,
        yn, ffp, 2 * l + 1, f'ln2b_{l}', tagp=f'l{pre}b')
    for b in range(BL):
        nc.vector.scalar_tensor_tensor(
            out=hn[b][:, 0:T], in0=ffp[b][:, 0:T], scalar=0.0,
            in1=yn[b][:, 0:T], op0=ALU.max, op1=ALU.add, accum_out=nsums[b])
    return hn, nsums


def _ln(nc, g, hp, hpt, small, tmp, psum, stats, colap, onesH, w_lng,
        y, out, grow, bname, tagp):
    """LayerNorm over the partition axis for each (batch, t) column.
    Stats rows live at partition 32*b of (128, T) tiles."""
    stat_y = stats.tile([128, T], BF16, tag='stat_y', name=f'sty_{tagp}')
    stat_q = stats.tile([128, T], BF16, tag='stat_q', name=f'stq_{tagp}')
    for n in range(NCH):
        p1 = psum.tile([128, CH], F32, tag='st', bufs=1, name=f'st1_{tagp}_{n}')
        p2 = psum.tile([128, CH], F32, tag='st2', bufs=1, name=f'st2_{tagp}_{n}')
        for b in range(BL):
            sq = tmp.tile([128, CH], BF16, tag='ln', bufs=3, name=f'sq{tagp}_{b}_{n}')
            nc.vector.tensor_tensor(out=sq, in0=y[b][:, CH * n:CH * (n + 1)],
                                    in1=y[b][:, CH * n:CH * (n + 1)],
                                    op=ALU.mult)
            nc.tensor.matmul(p1[32 * b:32 * b + 1, :], onesH,
                             y[b][:, CH * n:CH * (n + 1)],
                             tile_position=(0, 32 * b))
            nc.tensor.matmul(p2[32 * b:32 * b + 1, :], onesH, sq,
                             tile_position=(0, 32 * b))
        # mu, e2 (psum already scaled by 1/H via onesH)
        nc.vector.tensor_scalar(out=stat_y[:, CH * n:CH * (n + 1)], in0=p1,
                                scalar1=1.0, scalar2=None, op0=ALU.mult)
        nc.vector.tensor_scalar(out=stat_q[:, CH * n:CH * (n + 1)], in0=p2,
                                scalar1=float(EPS), scalar2=None, op0=ALU.add)
    # var+eps = (e2+eps) - mu^2; inv = sqrt(1/(var+eps)); negq = -mu*inv
    # (inv overwrites stat_q; the fp32 scratch is chunked to save SBUF)
    for n in range(NCH):
        cs = slice(CH * n, CH * (n + 1))
        sv32 = stats.tile([128, CH], F32, tag='sv32', bufs=2,
                          name=f'sv32_{tagp}_{n}')
        nc.gpsimd.tensor_tensor(out=sv32, in0=stat_y[:, cs],
                                in1=stat_y[:, cs], op=ALU.mult)
        nc.gpsimd.tensor_tensor(out=sv32, in0=stat_q[:, cs], in1=sv32,
                                op=ALU.subtract)
        nc.vector.reciprocal_approx_fast(out=sv32, in_=sv32)
        nc.scalar.activation(out=stat_q[:, cs], in_=sv32, func=AF.Sqrt)
    nc.vector.scalar_tensor_tensor(out=stat_y, in0=stat_y, scalar=-1.0,
                                   in1=stat_q, op0=ALU.mult, op1=ALU.mult)
    inv, negq = stat_q, stat_y
    bcol = colap(bname)
    for b in range(BL):
        pb = hpt(f'bcP{tagp}_{b}')
        qb = hpt(f'bcQ{tagp}_{b}')
        for n in range(NCH):
            bp = psum.tile([128, CH], F32, tag='st', bufs=1,
                           name=f'bp_{tagp}_{b}_{n}')
            bq = psum.tile([128, CH], F32, tag='st2', bufs=1,
                           name=f'bq_{tagp}_{b}_{n}')
            # pb = g[h] * inv[t]; qb = g[h] * negq[t] + beta[h]
            nc.tensor.matmul(bp, w_lng[32 * b:32 * b + 1, grow, :],
                             inv[32 * b:32 * b + 1, CH * n:CH * (n + 1)],
                             tile_position=(32 * b, 0))
            nc.tensor.matmul(bq, w_lng[32 * b:32 * b + 1, grow, :],
                             negq[32 * b:32 * b + 1, CH * n:CH * (n + 1)],
                             tile_position=(32 * b, 0))
            nc.scalar.activation(out=pb[:, CH * n:CH * (n + 1)], in_=bp,
                                 func=AF.Copy)
            nc.vector.tensor_scalar(out=qb[:, CH * n:CH * (n + 1)], in0=bq,
                                    scalar1=bcol, scalar2=None, op0=ALU.add)
        for n in range(NCH):
            tl = tmp.tile([128, CH], BF16, tag='ln', bufs=3, name=f'tl{tagp}_{b}_{n}')
            nc.vector.tensor_tensor(out=tl, in0=y[b][:, CH * n:CH * (n + 1)],
                                    in1=pb[:, CH * n:CH * (n + 1)],
                                    op=ALU.mult)
            nc.vector.tensor_tensor(out=out[b][:, CH * n:CH * (n + 1)],
                                    in0=tl, in1=qb[:, CH * n:CH * (n + 1)],
                                    op=ALU.add)


# ---------------------------------------------------------------------------
# entry point
# ---------------------------------------------------------------------------

_CACHE = {}


def kernel(**inputs):
    prep = _prep_weights(inputs)
    nv = prep['cvec'].shape[1]
    key = ('nc', prep['ln_trivial'])
    if key not in _CACHE:
        _CACHE[key] = _build_nc(nv, ln_trivial=prep['ln_trivial'])
    nc = _CACHE[key]
    _CACHE['nc'] = nc

    x = np.asarray(inputs['x'], np.float32).astype(bf16np)
    xc = x.transpose(0, 2, 1)                       # (B, C, T)
    xs = np.zeros((B, 64, TP + 1), dtype=bf16np)
    xs[:, :, 2:2 + T] = xc
    in_map = {
        'x': np.ascontiguousarray(xs),
        'cvec': prep['cvec'],
        'attn_rhs': prep['attn_rhs'], 'dupP': prep['dupP'],
        'band_lhsT': prep['band_lhsT'],
        'res_pair': prep['res_pair'], 'res_q2p': prep['res_q2p'],
        'res_q2s': prep['res_q2s'],
        'proj_pair': prep['proj_pair'], 'proj_sing': prep['proj_sing'],
        'rwkv_lhsT': prep['rwkv_lhsT'], 'lngain': prep['lngain'],
        'cls1_lhsT': prep['cls1_lhsT'], 'cls2_lhsT': prep['cls2_lhsT'],
    }
    _CACHE['in_maps'] = [in_map]
    res = run_bass_kernel_spmd(nc, [in_map], core_ids=[0])
    out = res.results[0]['out']                     # (NCLS, B)
    return np.ascontiguousarray(out.T).astype(np.float32)


def profile_exec_ns():
    if 'nc' not in _CACHE or 'in_maps' not in _CACHE:
        return None
    try:
        res = run_bass_kernel_spmd(_CACHE['nc'], _CACHE['in_maps'],
                                   core_ids=[0], trace=True)
    except Exception as e:
        print("profile unavailable:", e)
        return None
    if res.instructions_and_trace is not None:
        print("trace:", res.instructions_and_trace[1])
    return res.exec_time_ns


def bench_exec(n=8):
    """Steady-state timing of the compiled executable (device-resident
    inputs, jit built once). Returns (min_s, avg_s) per call."""
    import time
    import jax
    from jax.sharding import Mesh, PartitionSpec
    from jax.experimental.shard_map import shard_map
    from concourse import bass2jax as b2j
    from concourse import mybir

    nc = _CACHE['nc']; in_maps = _CACHE['in_maps']
    b2j.install_neuronx_cc_hook()
    partition_name = (nc.partition_id_tensor.name
                      if nc.partition_id_tensor else None)
    in_names, out_names, out_avals, zero_outs = [], [], [], []
    for alloc in nc.m.functions[0].allocations:
        if not isinstance(alloc, mybir.MemoryLocationSet):
            continue
        name = alloc.memorylocations[0].name
        if alloc.kind == 'ExternalInput':
            if name != partition_name:
                in_names.append(name)
        elif alloc.kind == 'ExternalOutput':
            sh = tuple(alloc.tensor_shape)
            dt = mybir.dt.np(alloc.dtype)
            out_avals.append(jax.core.ShapedArray(sh, dt))
            out_names.append(name)
            zero_outs.append(np.zeros(sh, dt))
    n_params = len(in_names)
    n_outs = len(out_avals)
    all_in_names = list(in_names) + list(out_names)
    if partition_name is not None:
        all_in_names.append(partition_name)

    def _body(*args):
        operands = list(args)
        if partition_name is not None:
            operands.append(b2j.partition_id_tensor())
        outs = b2j._bass_exec_p.bind(
            *operands, out_avals=tuple(out_avals),
            in_names=tuple(all_in_names),
            out_names=tuple(out_names), lowering_input_output_aliases=(),
            sim_require_finite=True, sim_require_nnan=True, nc=nc)
        return tuple(outs)

    devices = jax.devices()[:NCORE]
    mesh = Mesh(np.asarray(devices), ('core',))
    in_specs = (PartitionSpec('core'),) * (n_params + n_outs)
    out_specs = (PartitionSpec('core'),) * len(out_names)
    sharded = jax.jit(shard_map(_body, mesh=mesh, in_specs=in_specs,
                                out_specs=out_specs, check_rep=False),
                      keep_unused=True)
    concat_in = [np.concatenate([np.asarray(in_maps[c][nm])
                                 for c in range(NCORE)], axis=0)
                 for nm in in_names]
    concat_zeros = [np.zeros((NCORE * z.shape[0], *z.shape[1:]), z.dtype)
                    for z in zero_outs]
    args = [jax.device_put(a) for a in concat_in + concat_zeros]
    r = sharded(*args); jax.block_until_ready(r)   # warmup/compile
    def run_n(k):
        t0 = time.perf_counter()
        rs = [sharded(*args) for _ in range(k)]
        jax.block_until_ready(rs)
        return time.perf_counter() - t0
    run_n(2)
    t1 = min(run_n(1) for _ in range(3))
    tn = min(run_n(n) for _ in range(3))
    slope = (tn - t1) / (n - 1)
    return t1, slope

